# revision 2
# baseline (speedup 1.0000x reference)
"""Trainium2 Bass kernel for EvolveGCN-O forward (GCN message passing).

Math (reference):
    h   = x @ Wp + bp
    W   = LSTM-evolved weight from initial_weight (one step, h0=c0=IW)
    hw  = h @ W
    out = D^-1/2 (A+I) D^-1/2 hw + b_gcn

Factored for the kernel:
    out[d] = dinv[d] * (sum_{e: dst=d} dinv[src_e] * x[src_e]) @ (Wp @ W)
             + s2[d]*dinv[d]*(bp @ W) + b_gcn
with s2[d] = sum_{e in(d)} dinv[src_e] (self loops included as edges).

Distribution: dst nodes sharded contiguously over 8 NeuronCores (1280 each).
The aggregation sum_{e:dst=d} dinv[src]*x[src] is computed as dense matmuls
against a per-core count matrix S[src, dst_local] (fp8, exact small ints;
self loops folded in):  xagg[feat, dst] = sum_r xs_r^T @ S_r  over the 80
src ranks of 128.  S (13.1MB/core) streams from HBM on the sync+scalar
HWDGE queues at full descriptor width while the TensorEngine consumes it;
no SWDGE gather and no gpsimd library swap.  Column passes of 512/512/256
dsts let early groups' finals overlap later passes.  The tiny evolved-
weight LSTM is replicated on every core.
"""

import numpy as np

N_NODES = 10000
N_EDGES = 320000
IN_DIM = 128
HID = 256
M = 8                    # NeuronCores
NP = 10240               # padded node count (mult of 128)
RANKS = NP // 128        # 80 src ranks
NPC = NP // M            # 1280 dsts per core
NGRP = NPC // 128        # 10 dst groups of 128 per core
CHUNK = 4                # src ranks per S-stream DMA
NCHUNK = RANKS // CHUNK  # 20
XHEAD = 8                # ranks in the first (fast) xs load
PASSES = [(0, 512), (512, 512), (1024, 256)]   # dst col tiles
GATES_AFTER = 8          # emit LSTM gate matmuls after this many ranks of pass A

_cache = {}


def _build_module():
    import concourse.bacc as bacc
    import concourse.mybir as mybir
    import concourse.tile as tile

    nc = bacc.Bacc("TRN2", target_bir_lowering=False, debug=False,
                   num_devices=M)
    f32, f16, f8 = mybir.dt.float32, mybir.dt.float16, mybir.dt.float8e4

    # ---- DRAM inputs ----
    s_in = nc.dram_tensor("S", [128, RANKS * NPC], f8, kind="ExternalInput").ap()
    xs_in = nc.dram_tensor("xs_tiled", [128, RANKS * 128], f16, kind="ExternalInput").ap()
    wsum_in = nc.dram_tensor("wsumT", [256, 1024], f16, kind="ExternalInput").ap()
    bsum_in = nc.dram_tensor("bsum", [1, 1024], f16, kind="ExternalInput").ap()
    iw_in = nc.dram_tensor("IW", [256, 256], f32, kind="ExternalInput").ap()
    iwt_in = nc.dram_tensor("IWT", [256, 256], f16, kind="ExternalInput").ap()
    wpt_in = nc.dram_tensor("WpT", [256, 128], f16, kind="ExternalInput").ap()
    bp_in = nc.dram_tensor("bp_col", [256, 1], f16, kind="ExternalInput").ap()
    bgcn_in = nc.dram_tensor("b_gcn", [1, 256], f16, kind="ExternalInput").ap()
    ones_in = nc.dram_tensor("ones_row", [1, 128], f16, kind="ExternalInput").ap()
    s2_in = nc.dram_tensor("s2_row", [1, NPC], f16, kind="ExternalInput").ap()
    dri_in = nc.dram_tensor("dri_row", [1, NPC], f16, kind="ExternalInput").ap()
    dcol_in = nc.dram_tensor("dinv_col", [128, NGRP], f32, kind="ExternalInput").ap()

    out_t = nc.dram_tensor("out", [NPC, HID], f32, kind="ExternalOutput").ap()

    Sig = mybir.ActivationFunctionType.Sigmoid
    Tanh = mybir.ActivationFunctionType.Tanh

    with tile.TileContext(nc) as tc:
        with (
            tc.tile_pool(name="persist", bufs=1) as pp,
            tc.tile_pool(name="schunks", bufs=NCHUNK) as spool,
            tc.tile_pool(name="stage", bufs=1) as stpool,
            tc.tile_pool(name="fin", bufs=4) as fpool,
            tc.tile_pool(name="psacc", bufs=2, space="PSUM") as psacc,
            tc.tile_pool(name="psg", bufs=2, space="PSUM") as psg,
            tc.tile_pool(name="psl", bufs=1, space="PSUM") as psl,
        ):
            # ---------- input DMAs ----------
            # sync:   S0, S2, ... (+ out writes later)
            # scalar: xs_head, xs_tail, S1, S3, ...
            # gpsimd: all small weight tensors
            xs_head = pp.tile([128, XHEAD, 128], f16)
            xs_tail = pp.tile([128, RANKS - XHEAD, 128], f16)
            xs_r = xs_in.rearrange("p (r f) -> p r f", f=128)
            nc.scalar.dma_start(out=xs_head[:], in_=xs_r[:, 0:XHEAD, :])
            nc.scalar.dma_start(out=xs_tail[:], in_=xs_r[:, XHEAD:RANKS, :])

            schunks = []
            s_r = s_in.rearrange("p (k c) -> p k c", c=CHUNK * NPC)
            for k in range(NCHUNK):
                sch = spool.tile([128, CHUNK, NPC], f8, tag="schunk",
                                 name=f"schunk{k}")
                eng = nc.sync if k % 2 == 0 else nc.scalar
                eng.dma_start(
                    out=sch[:],
                    in_=s_r[:, k, :].rearrange("p (j c) -> p j c", c=NPC),
                )
                schunks.append(sch)

            wsum = pp.tile([128, 2, 1024], f16)
            iwt = pp.tile([128, 2, 256], f16)
            iw = pp.tile([128, 2, 256], f32)
            wpt = pp.tile([128, 2, 128], f16)
            bp_c = pp.tile([128, 2, 1], f16)
            bsum = pp.tile([1, 1024], f16)
            bgcn = pp.tile([1, 256], f16)
            ones = pp.tile([1, 128], f16)
            s2r = pp.tile([1, NPC], f16)
            drir = pp.tile([1, NPC], f16)
            dcol = pp.tile([128, NGRP], f32)
            for t_, src_ in ((iwt, iwt_in), (wsum, wsum_in), (iw, iw_in),
                             (wpt, wpt_in), (bp_c, bp_in)):
                nc.gpsimd.dma_start(
                    out=t_[:], in_=src_.rearrange("(k p) c -> p k c", p=128))
            for t_, src_ in ((bsum, bsum_in), (bgcn, bgcn_in), (ones, ones_in),
                             (s2r, s2_in), (drir, dri_in), (dcol, dcol_in)):
                nc.gpsimd.dma_start(out=t_[:], in_=src_[:])

            # ---------- LSTM weight evolution (emitted mid pass A) ----------
            w_ev = pp.tile([128, 2, 256], f16)   # evolved GCN weight W

            def emit_gates():
                gpsum = psl.tile([128, 2, 1024], f32, space="PSUM", tag="gates")
                for ic in range(2):
                    for h in range(2):
                        gs = slice(512 * h, 512 * (h + 1))
                        nc.tensor.matmul(out=gpsum[:, ic, gs], lhsT=ones[:, :],
                                         rhs=bsum[:, gs], start=True, stop=False)
                        nc.tensor.matmul(out=gpsum[:, ic, gs],
                                         lhsT=iwt[:, 0, 128 * ic:128 * (ic + 1)],
                                         rhs=wsum[:, 0, gs], start=False, stop=False)
                        nc.tensor.matmul(out=gpsum[:, ic, gs],
                                         lhsT=iwt[:, 1, 128 * ic:128 * (ic + 1)],
                                         rhs=wsum[:, 1, gs], start=False, stop=True)
                return gpsum

            def emit_lstm_post(gpsum):
                for ic in range(2):
                    si = stpool.tile([128, 256], f32, tag="si", name=f"si{ic}")
                    sf = stpool.tile([128, 256], f32, tag="sf", name=f"sf{ic}")
                    so = stpool.tile([128, 256], f32, tag="so", name=f"so{ic}")
                    tg = stpool.tile([128, 256], f32, tag="tg", name=f"tg{ic}")
                    nc.scalar.activation(out=si[:], in_=gpsum[:, ic, 0:256], func=Sig)
                    nc.scalar.activation(out=sf[:], in_=gpsum[:, ic, 256:512], func=Sig)
                    nc.scalar.activation(out=so[:], in_=gpsum[:, ic, 768:1024], func=Sig)
                    nc.scalar.activation(out=tg[:], in_=gpsum[:, ic, 512:768], func=Tanh)
                    c1 = stpool.tile([128, 256], f32, tag="c1", name=f"c1_{ic}")
                    nc.vector.tensor_tensor(out=c1[:], in0=sf[:], in1=iw[:, ic, :],
                                            op=mybir.AluOpType.mult)
                    c2 = stpool.tile([128, 256], f32, tag="c2", name=f"c2_{ic}")
                    nc.vector.tensor_tensor(out=c2[:], in0=si[:], in1=tg[:],
                                            op=mybir.AluOpType.mult)
                    cc = stpool.tile([128, 256], f32, tag="cc", name=f"cc{ic}")
                    nc.vector.tensor_tensor(out=cc[:], in0=c1[:], in1=c2[:],
                                            op=mybir.AluOpType.add)
                    tcc = stpool.tile([128, 256], f32, tag="tcc", name=f"tcc{ic}")
                    nc.scalar.activation(out=tcc[:], in_=cc[:], func=Tanh)
                    nc.vector.tensor_tensor(out=w_ev[:, ic, :], in0=so[:],
                                            in1=tcc[:], op=mybir.AluOpType.mult)

            wpw = pp.tile([128, 256], f16)       # Wp @ W
            bpw = pp.tile([1, 256], f16)         # bp @ W

            def emit_wpw():
                wp_ps = psg.tile([128, HID], f32, space="PSUM", tag="ops",
                                 name="wp_ps")
                nc.tensor.matmul(out=wp_ps[:], lhsT=wpt[:, 0, :], rhs=w_ev[:, 0, :],
                                 start=True, stop=False)
                nc.tensor.matmul(out=wp_ps[:], lhsT=wpt[:, 1, :], rhs=w_ev[:, 1, :],
                                 start=False, stop=True)
                nc.vector.tensor_copy(out=wpw[:], in_=wp_ps[:])
                bp_ps = psg.tile([128, HID], f32, space="PSUM", tag="ops",
                                 name="bp_ps")
                nc.tensor.matmul(out=bp_ps[0:1, :], lhsT=bp_c[:, 0, :],
                                 rhs=w_ev[:, 0, :], start=True, stop=False)
                nc.tensor.matmul(out=bp_ps[0:1, :], lhsT=bp_c[:, 1, :],
                                 rhs=w_ev[:, 1, :], start=False, stop=True)
                nc.vector.tensor_copy(out=bpw[:], in_=bp_ps[0:1, :])

            def emit_final(g, xagg):
                ops = psg.tile([128, HID], f32, space="PSUM", tag="ops",
                               name=f"ops{g}")
                ds = slice(128 * g, 128 * (g + 1))
                nc.tensor.matmul(out=ops[:], lhsT=s2r[:, ds], rhs=bpw[:],
                                 start=True, stop=False)
                nc.tensor.matmul(out=ops[:], lhsT=drir[:, ds], rhs=bgcn[:],
                                 start=False, stop=False)
                nc.tensor.matmul(out=ops[:], lhsT=xagg[:], rhs=wpw[:],
                                 start=False, stop=True)
                orow = fpool.tile([128, HID], f32, tag="orow", name=f"orow{g}")
                nc.scalar.activation(out=orow[:], in_=ops[:],
                                     func=mybir.ActivationFunctionType.Copy,
                                     scale=dcol[:, g:g + 1])
                nc.sync.dma_start(
                    out=out_t.rearrange("(g p) h -> g p h", p=128)[g],
                    in_=orow[:],
                )

            # ---------- main: 3 column passes over all 80 src ranks ----------
            gpsum = None
            for pi, (c0, w) in enumerate(PASSES):
                acc = psacc.tile([128, w], f32, space="PSUM", tag="acc",
                                 name=f"acc{pi}")
                for r in range(RANKS):
                    if pi == 0 and r == GATES_AFTER:
                        gpsum = emit_gates()
                    xs_t = (xs_head[:, r, :] if r < XHEAD
                            else xs_tail[:, r - XHEAD, :])
                    sch = schunks[r // CHUNK]
                    nc.tensor.matmul(
                        out=acc[:],
                        lhsT=xs_t,
                        rhs=sch[:, r % CHUNK, c0:c0 + w],
                        start=(r == 0),
                        stop=(r == RANKS - 1),
                    )
                if pi == 0:
                    emit_lstm_post(gpsum)
                    emit_wpw()
                # finals for this pass's dst groups
                for gg in range(w // 128):
                    g = c0 // 128 + gg
                    xagg = fpool.tile([128, 128], f16, tag="xagg",
                                      name=f"xagg{g}")
                    nc.vector.tensor_copy(
                        out=xagg[:], in_=acc[:, 128 * gg:128 * (gg + 1)])
                    emit_final(g, xagg)

    nc.compile()
    return nc


def _preprocess(edge_index):
    """Host-side: degree norms, per-core fp8 count matrices (self loops in)."""
    import ml_dtypes

    src = np.asarray(edge_index[0], dtype=np.int64)
    dst = np.asarray(edge_index[1], dtype=np.int64)
    deg = np.bincount(dst, minlength=N_NODES).astype(np.float64) + 1.0
    dinv = (1.0 / np.sqrt(deg)).astype(np.float32)

    # s2[d] = sum over in-edges of dinv[src], self loop included
    s2 = (np.bincount(dst, weights=dinv[src].astype(np.float64),
                      minlength=N_NODES) + dinv.astype(np.float64)).astype(np.float32)

    core = dst // NPC
    dloc = dst - core * NPC
    s_mats = []
    for c in range(M):
        m = core == c
        flat = src[m] * NPC + dloc[m]
        cnt = np.bincount(flat, minlength=NP * NPC)
        # self loops of this core's nodes
        d0, d1 = c * NPC, min((c + 1) * NPC, N_NODES)
        dd = np.arange(d0, d1, dtype=np.int64)
        cnt[dd * NPC + (dd - d0)] += 1
        sc = cnt.reshape(RANKS, 128, NPC).transpose(1, 0, 2)
        s_mats.append(np.ascontiguousarray(sc).astype(ml_dtypes.float8_e4m3)
                      .reshape(128, RANKS * NPC))
    return dinv, s2, s_mats


LAST_RESULT = None


def kernel(x, edge_index, Wp, bp, W_ih, W_hh, b_ih, b_hh, initial_weight, b_gcn):
    global LAST_RESULT
    from concourse.bass_utils import run_bass_kernel_spmd

    x = np.asarray(x, np.float32)
    Wp = np.asarray(Wp, np.float32)
    bp = np.asarray(bp, np.float32)
    W_ih = np.asarray(W_ih, np.float32)
    W_hh = np.asarray(W_hh, np.float32)
    b_ih = np.asarray(b_ih, np.float32)
    b_hh = np.asarray(b_hh, np.float32)
    initial_weight = np.asarray(initial_weight, np.float32)
    b_gcn = np.asarray(b_gcn, np.float32)
    assert x.shape == (N_NODES, IN_DIM)

    dinv, s2, s_mats = _preprocess(edge_index)

    if "nc" not in _cache:
        _cache["nc"] = _build_module()
    nc = _cache["nc"]

    # host pre-scales x rows by dinv[src]; fp16 token table
    xp = np.zeros((NP, IN_DIM), np.float32)
    xp[:N_NODES] = x * dinv[:, None]
    xs_tiled = np.ascontiguousarray(
        xp.reshape(RANKS, 128, IN_DIM).transpose(1, 0, 2)
        .reshape(128, RANKS * 128)).astype(np.float16)

    wsumT = np.ascontiguousarray((W_ih + W_hh).T).astype(np.float16)
    bsum = (b_ih + b_hh).reshape(1, -1).astype(np.float16)

    s2p = np.zeros(NP, np.float32)
    s2p[:N_NODES] = s2
    drip = np.zeros(NP, np.float32)
    drip[:N_NODES] = 1.0 / dinv
    dlocp = np.zeros(NP, np.float32)
    dlocp[:N_NODES] = dinv

    shared = {
        "xs_tiled": xs_tiled,
        "wsumT": wsumT,
        "bsum": bsum,
        "IW": initial_weight,
        "IWT": np.ascontiguousarray(initial_weight.T).astype(np.float16),
        "WpT": np.ascontiguousarray(Wp.T).astype(np.float16),
        "bp_col": np.ascontiguousarray(bp.reshape(-1, 1)).astype(np.float16),
        "b_gcn": b_gcn.reshape(1, -1).astype(np.float16),
        "ones_row": np.ones((1, 128), np.float16),
    }
    in_maps = []
    for c in range(M):
        sl = slice(c * NPC, (c + 1) * NPC)
        in_maps.append({
            **shared,
            "S": s_mats[c],
            "s2_row": s2p[sl].reshape(1, -1).astype(np.float16),
            "dri_row": drip[sl].reshape(1, -1).astype(np.float16),
            "dinv_col": np.ascontiguousarray(
                dlocp[sl].reshape(NGRP, 128).T),
        })

    res = run_bass_kernel_spmd(nc, in_maps, list(range(M)))
    LAST_RESULT = res

    out = np.empty((N_NODES, HID), np.float32)
    for c in range(M):
        d0, d1 = c * NPC, min((c + 1) * NPC, N_NODES)
        out[d0:d1] = res.results[c]["out"][:d1 - d0]
    return out
